# revision 3
# baseline (speedup 1.0000x reference)
"""Causal self-attention (B=1, T=4096, E=1024, H=16, D=64) on 8 TRN2 NeuronCores.

Sharding: tensor-parallel over heads — each core owns 2 heads (128 of the
1024 hidden dims). Each core computes its slice of the QKV projection, a
flash-style causal attention for its 2 heads, and a partial output
projection (rows of w_out for its head dims). The host sums the 8 partial
outputs (the row-parallel all-reduce) and adds b_out.

Matmul inputs are bf16 (1 cycle/row on the PE), accumulation fp32 in PSUM.
x and weights are converted to bf16 on the host (halves their DMA too).

The per-head d=64 S^T matmuls use 64x128 PE row tiling: head 0 runs on
array rows 0-63 (tile (0,0)) while head 1 runs concurrently on rows 64-127
(tile (64,0)), each contracting its own K=64 head dims. The two MMs issue
back-to-back and stream in parallel, so a [tk=128, 2, 512] S^T block costs
~512 PE cycles instead of the 1024 a zero-padded K=128 packing costs.
qT/kT/vT all live in the natural [128 = 2 heads x 64 dims, T] layout.

Per-core dataflow (feature-major throughout; tq blocks of 512; tk blocks
of 128):
  kT/vT/qT [128, 4096]  (phase A, K=e chunks of 128)
  V' [tk, 2, 65] = PE-transpose of vT + ones column
  per (512-wide tq block qb):
    per tk block tb:   S^T[tk, h, f] row-tiled pair     [128, 2, 512] PSUM
                       P = exp(0.125 * S^T)             ACT, PSUM->SBUF bf16
                       (diag blocks: affine_select zeroes tq < tk)
      per head h:      O'_h += V'_h.T @ P[:, h, :]      [65, 512] PSUM accum
    row 64 of O'_h = softmax denominators (ones column trick);
    normalize: broadcast denom row, fast reciprocal, columnwise scale
    -> UnT [hd=128, t] (both heads stacked)
  out_partial[t,:] = UnT_tile.T @ w_out_rows  (pipelined one qb behind,
  bf16 partials; host upcasts and sums)
"""

import sys

for _p in ("/opt/trn_rl_repo",):
    if _p not in sys.path:
        sys.path.insert(0, _p)

import ml_dtypes
import numpy as np

import concourse.bass as bass  # noqa: F401
import concourse.mybir as mybir
import concourse.tile as tile
from concourse import bacc
from concourse.bass_utils import run_bass_kernel_spmd
from concourse.masks import make_identity

T, E = 4096, 1024
H, D = 16, 64
NCORES = 8
HPC = H // NCORES          # heads per core = 2
HD = HPC * D               # hidden dims per core = 128
NT = T // 512              # 8 tq blocks of 512
NE = E // 128              # 8 e-chunks of 128
NTB = T // 128             # 32 tk blocks of 128

F32 = mybir.dt.float32
BF16 = mybir.dt.bfloat16
NPBF16 = np.dtype(ml_dtypes.bfloat16)
AF = mybir.ActivationFunctionType


def _build_kernel():
    nc = bacc.Bacc("TRN2", target_bir_lowering=False, debug=False)

    xT = nc.dram_tensor("xT", [E, T], BF16, kind="ExternalInput")
    wq = nc.dram_tensor("wq", [E, HD], BF16, kind="ExternalInput")
    wk = nc.dram_tensor("wk", [E, HD], BF16, kind="ExternalInput")
    wv = nc.dram_tensor("wv", [E, HD], BF16, kind="ExternalInput")
    bqkv = nc.dram_tensor("bqkv", [3, HD, 1], F32, kind="ExternalInput")
    wo = nc.dram_tensor("wo", [HD, E], BF16, kind="ExternalInput")
    out = nc.dram_tensor("out", [T, E], BF16, kind="ExternalOutput")

    with tile.TileContext(nc) as tc:
        _body(nc, tc, xT, wq, wk, wv, bqkv, wo, out)
    nc.compile()
    return nc


def _body(nc, tc, xT, wq, wk, wv, bqkv, wo, out):
    from contextlib import ExitStack

    ctx = ExitStack()
    with ctx:
        const = ctx.enter_context(tc.tile_pool(name="const", bufs=1))
        big = ctx.enter_context(tc.tile_pool(name="big", bufs=1))
        xpool = ctx.enter_context(tc.tile_pool(name="xp", bufs=3))
        ppool = ctx.enter_context(tc.tile_pool(name="pp", bufs=4))
        opool = ctx.enter_context(tc.tile_pool(name="op", bufs=3))
        small = ctx.enter_context(tc.tile_pool(name="sm", bufs=4))
        ps_mm = ctx.enter_context(tc.tile_pool(name="ps_mm", bufs=2, space="PSUM"))
        ps_o = ctx.enter_context(tc.tile_pool(name="ps_o", bufs=2, space="PSUM"))
        ps_q = ctx.enter_context(tc.tile_pool(name="ps_q", bufs=2, space="PSUM"))

        # ---- constants / weights ----
        # scratch read by the clock-warming matmuls; one cheap memset is
        # its only producer so the warm-up can start immediately
        warm_src = const.tile([128, 512], BF16)
        nc.vector.memset(warm_src[:], 0.0)
        identb = const.tile([128, 128], BF16)
        make_identity(nc, identb[:])

        xs_map = {}

        def load_x(tcc):
            # one DMA for the whole [1024, 512] chunk (8 e-slices)
            ts512 = slice(tcc * 512, (tcc + 1) * 512)
            xsb = xpool.tile([128, NE, 512], BF16, tag="xsb")
            nc.sync.dma_start(
                xsb[:], xT[:, ts512].rearrange("(a p) t -> p a t", p=128)
            )
            xs_map[tcc] = xsb

        # x chunk 0 first: it gates the first QKV matmuls, while the
        # weight DMAs can hide under the warm-up/projection stream
        load_x(0)

        wq_sb = const.tile([128, NE, HD], BF16)
        wk_sb = const.tile([128, NE, HD], BF16)
        wv_sb = const.tile([128, NE, HD], BF16)
        for w_dram, w_sb in ((wq, wq_sb), (wk, wk_sb), (wv, wv_sb)):
            nc.sync.dma_start(
                w_sb[:], w_dram[:].rearrange("(a p) c -> p a c", p=128)
            )
        bq_sb = const.tile([128, 1], F32)
        bk_sb = const.tile([128, 1], F32)
        bv_sb = const.tile([128, 1], F32)
        nc.sync.dma_start(bq_sb[:], bqkv[0])
        nc.sync.dma_start(bk_sb[:], bqkv[1])
        nc.sync.dma_start(bv_sb[:], bqkv[2])

        load_x(1)
        wo_sb = const.tile([128, E], BF16)
        nc.sync.dma_start(wo_sb[:], wo[:])

        qT = big.tile([128, T], BF16)
        kT = big.tile([128, T], BF16)
        vT = big.tile([128, T], BF16)
        # V row-major per (tk block, head) plus a ones column.
        V2 = big.tile([128, NTB, HPC, D + 1], BF16)
        # normalized attention outputs, transposed: rows h*64+d, cols t
        UnT = big.tile([128, T], BF16)

        nc.gpsimd.memset(V2[:, :, :, D], 1.0)

        wparams = ((wq_sb, bq_sb), (wk_sb, bk_sb), (wv_sb, bv_sb))
        qkv_dst = (qT, kT, vT)

        def emit_qkv(tcc, m):
            w_sb, b_sb = wparams[m]
            ts512 = slice(tcc * 512, (tcc + 1) * 512)
            ps = ps_q.tile([128, 512], F32, tag="q")
            for ec in range(NE):
                nc.tensor.matmul(
                    ps[:], w_sb[:, ec, :], xs_map[tcc][:, ec, :],
                    start=(ec == 0), stop=(ec == NE - 1),
                )
            nc.vector.tensor_scalar_add(qkv_dst[m][:, ts512], ps[:], b_sb[:])

        def emit_vtrans(tcc, j):
            # V' transpose, K=128 full-width: both heads in one go
            tb = 4 * tcc + j
            pst = ps_q.tile([128, 128], BF16, tag="q")
            nc.tensor.transpose(
                pst[:], vT[:, tb * 128:(tb + 1) * 128], identb[:]
            )
            nc.vector.tensor_copy(
                V2[:, tb, :, 0:D],
                pst[:].rearrange("p (h d) -> p h d", h=HPC),
            )

        def emit_piece(piece):
            kind = piece[0]
            if kind == "qkv":
                emit_qkv(piece[1], piece[2])
            elif kind == "vtrans":
                emit_vtrans(piece[1], piece[2])
            else:
                _outproj_tile(nc, ps_mm, opool, UnT, wo_sb, out, piece[1])

        def emit_S(qb, tb):
            # row-tiled pair: head h contracts its own 64 dims on array
            # rows h*64..h*64+63; both MMs stream concurrently
            f0 = max(0, tb * 128 - qb * 512)
            psS = ps_mm.tile([128, HPC, 512], F32, tag="mm")
            t0 = qb * 512 + f0
            t1 = (qb + 1) * 512
            for h in range(HPC):
                nc.tensor.matmul(
                    psS[:, h, f0:512],
                    kT[h * D:(h + 1) * D, tb * 128:(tb + 1) * 128],
                    qT[h * D:(h + 1) * D, t0:t1],
                    start=True, stop=True,
                )
            return psS

        # ---- prologue: x for chunks 0/1, full QKV chunk 0. Dummy
        # identity matmuls fill the input-DMA wait: they cost nothing the
        # PE would otherwise use, and ~4us of sustained full-array
        # activity flips the clock gate to 2.4 GHz before the real
        # matmuls start ----
        for i in range(10):
            wps = ps_q.tile([128, 512], F32, tag="q")
            nc.tensor.matmul(wps[:], warm_src[:, 0:128], warm_src[:],
                             start=True, stop=True)
        for m in range(3):
            emit_qkv(0, m)
        for j in range(4):
            emit_vtrans(0, j)

        # ---- merged pipeline: per step, the attention q block `step`
        # runs its ACT-paced tk loop while the NEXT chunk's projection
        # matmuls and the lag-1 out-proj are spread through it as PE
        # filler pieces ----
        for step in range(NT):
            if step + 2 < NT:
                load_x(step + 2)
            pieces = []
            if step + 1 < NT:
                pieces += [("qkv", step + 1, m) for m in range(3)]
                pieces += [("vtrans", step + 1, j) for j in range(4)]
            if step >= 1:
                # lag-1: block step-1's normalize settles a few tk
                # iterations into this step
                pieces += [("out", tt)
                           for tt in range((step - 1) * 4, step * 4)]
            qb = step
            nblk = 4 * (qb + 1)
            emit_at = {}
            for i, piece in enumerate(pieces):
                emit_at.setdefault((i + 1) * nblk // (len(pieces) + 1),
                                   []).append(piece)
            pos = []
            for h in range(HPC):
                po = ps_o.tile([D + 1, 512], F32, tag="o")
                pos.append(po)

            def emit_O(tb, P):
                f0 = max(0, tb * 128 - qb * 512)
                for h in range(HPC):
                    nc.tensor.matmul(
                        pos[h][:, f0:512],
                        V2[:, tb, h, :],
                        P[:, h, f0:512],
                        start=(tb == 0), stop=(tb == nblk - 1),
                    )

            Stiles = {0: emit_S(qb, 0)}
            Pprev = None
            for tb in range(nblk):
                diag = tb >= 4 * qb
                f0 = max(0, tb * 128 - qb * 512)
                psS = Stiles.pop(tb)
                P = ppool.tile([128, HPC, 512], BF16, tag="P")
                nc.scalar.activation(
                    P[:, :, f0:512], psS[:, :, f0:512], AF.Exp, scale=0.125
                )
                if diag:
                    # keep where tq >= tk:
                    # (qb*512 + f0 + f') - (tb*128 + p) >= 0
                    nc.gpsimd.affine_select(
                        out=P[:, :, f0:512], in_=P[:, :, f0:512],
                        compare_op=mybir.AluOpType.is_ge,
                        fill=0.0,
                        base=qb * 512 + f0 - tb * 128,
                        channel_multiplier=-1,
                        pattern=[[0, HPC], [1, 512 - f0]],
                    )
                # PE stream per iteration: S(tb+1), filler pieces, then
                # O'(tb-1) — whose exp finished an iteration ago, so the
                # PE never sits out an exp latency
                if tb + 1 < nblk:
                    Stiles[tb + 1] = emit_S(qb, tb + 1)
                for piece in emit_at.get(tb, ()):
                    emit_piece(piece)
                if Pprev is not None:
                    emit_O(tb - 1, Pprev)
                Pprev = P
            emit_O(nblk - 1, Pprev)
            # normalize: U = O'[0:64] * (1 / O'[64]) columnwise; the two
            # heads' chains are interleaved by stage so DVE and GpSimd
            # pipeline them instead of running them back-to-back
            drows, rbs, rbrs = [], [], []
            for h in range(HPC):
                drow = small.tile([1, 512], F32, tag="drow")
                nc.vector.tensor_copy(drow[:], pos[h][D:D + 1, :])
                drows.append(drow)
            for h in range(HPC):
                rb = small.tile([D, 512], F32, tag="rb")
                nc.gpsimd.partition_broadcast(rb[:], drows[h][:], channels=D)
                rbs.append(rb)
            for h in range(HPC):
                rbr = small.tile([D, 512], F32, tag="rbr")
                nc.vector.reciprocal_approx_fast(rbr[:], rbs[h][:])
                rbrs.append(rbr)
            for h in range(HPC):
                nc.vector.tensor_mul(
                    UnT[h * D:(h + 1) * D, qb * 512:(qb + 1) * 512],
                    pos[h][0:D, :], rbrs[h][:],
                )
        for tt in range((NT - 1) * 4, NT * 4):
            _outproj_tile(nc, ps_mm, opool, UnT, wo_sb, out, tt)


def _outproj_tile(nc, ps_mm, opool, UnT, wo_sb, out, tt):
    osb2 = opool.tile([128, E], BF16, tag="out")
    for half in range(2):
        psc = ps_mm.tile([128, HPC, 512], F32, tag="mm")
        nc.tensor.matmul(
            psc[:, 0, :],
            UnT[:, tt * 128:(tt + 1) * 128],
            wo_sb[:, half * 512:(half + 1) * 512],
            start=True, stop=True,
        )
        nc.vector.tensor_copy(
            osb2[:, half * 512:(half + 1) * 512], psc[:, 0, :]
        )
    nc.sync.dma_start(out[tt * 128:(tt + 1) * 128, :], osb2[:])


_NC_CACHE = None


def _get_nc():
    global _NC_CACHE
    if _NC_CACHE is None:
        _NC_CACHE = _build_kernel()
    return _NC_CACHE


def _make_in_maps(x, w_qkv, b_qkv, w_out):
    x2 = np.asarray(x, dtype=np.float32).reshape(T, E)
    xT = np.ascontiguousarray(x2.T).astype(NPBF16)
    w_qkv = np.asarray(w_qkv, dtype=np.float32)
    b_qkv = np.asarray(b_qkv, dtype=np.float32)
    w_out = np.asarray(w_out, dtype=np.float32)
    in_maps = []
    for c in range(NCORES):
        s = slice(c * HD, (c + 1) * HD)
        in_maps.append({
            "xT": xT,
            "wq": np.ascontiguousarray(
                w_qkv[:, 0 * E + c * HD:0 * E + (c + 1) * HD]).astype(NPBF16),
            "wk": np.ascontiguousarray(
                w_qkv[:, 1 * E + c * HD:1 * E + (c + 1) * HD]).astype(NPBF16),
            "wv": np.ascontiguousarray(
                w_qkv[:, 2 * E + c * HD:2 * E + (c + 1) * HD]).astype(NPBF16),
            "bqkv": np.ascontiguousarray(
                np.stack([
                    b_qkv[0 * E + c * HD:0 * E + (c + 1) * HD],
                    b_qkv[1 * E + c * HD:1 * E + (c + 1) * HD],
                    b_qkv[2 * E + c * HD:2 * E + (c + 1) * HD],
                ]).reshape(3, HD, 1)
            ),
            "wo": np.ascontiguousarray(w_out[s, :]).astype(NPBF16),
        })
    return in_maps


def run_sharded(x, w_qkv, b_qkv, w_out, b_out, trace=False):
    """Run the SPMD kernel; returns (full_output, BassKernelResults)."""
    nc = _get_nc()
    in_maps = _make_in_maps(x, w_qkv, b_qkv, w_out)
    res = run_bass_kernel_spmd(
        nc, in_maps, core_ids=list(range(NCORES)), trace=trace
    )
    acc = np.zeros((T, E), dtype=np.float32)
    for c in range(NCORES):
        acc += np.asarray(res.results[c]["out"], dtype=np.float32)
    acc += np.asarray(b_out, dtype=np.float32)[None, :]
    return acc.reshape(1, T, E), res


def kernel(x, w_qkv, b_qkv, w_out, b_out):
    out, _ = run_sharded(x, w_qkv, b_qkv, w_out, b_out, trace=False)
    return out


# revision 10
# speedup vs baseline: 1.1731x; 1.1731x over previous
"""Causal self-attention (B=1, T=4096, E=1024, H=16, D=64) on 8 TRN2 NeuronCores.

Sharding: tensor-parallel over heads — each core owns 2 heads (128 of the
1024 hidden dims). Each core computes its slice of the QKV projection, a
flash-style causal attention for its 2 heads, and a partial output
projection (rows of w_out for its head dims). The host sums the 8 partial
outputs (the row-parallel all-reduce) and adds b_out.

Matmul inputs are bf16 (1 cycle/row on the PE), accumulation fp32 in PSUM.
x and weights are converted to bf16 on the host (halves their DMA too).

The per-head d=64 S^T matmuls use 64x128 PE row tiling: head 0 runs on
array rows 0-63 (tile (0,0)) while head 1 runs concurrently on rows 64-127
(tile (64,0)), each contracting its own K=64 head dims. The two MMs issue
back-to-back and stream in parallel, so a [tk=128, 2, 512] S^T block costs
~512 PE cycles instead of the 1024 a zero-padded K=128 packing costs.
qT/kT/vT all live in the natural [128 = 2 heads x 64 dims, T] layout.

Per-core dataflow (feature-major throughout; tq blocks of 512; tk blocks
of 128):
  kT/vT/qT [128, 4096]  (phase A, K=e chunks of 128)
  V' [tk, 2, 65] = PE-transpose of vT + ones column
  per (512-wide tq block qb):
    per tk block tb:   S^T[tk, h, f] row-tiled pair     [128, 2, 512] PSUM
                       P = exp(0.125 * S^T)             ACT, PSUM->SBUF bf16
                       (diag blocks: affine_select zeroes tq < tk)
      per head h:      O'_h += V'_h.T @ P[:, h, :]      [65, 512] PSUM accum
    row 64 of O'_h = softmax denominators (ones column trick);
    normalize: broadcast denom row, fast reciprocal, columnwise scale
    -> UnT [hd=128, t] (both heads stacked)
  out_partial[t,:] = UnT_tile.T @ w_out_rows  (pipelined one qb behind,
  bf16 partials; host upcasts and sums)
"""

import sys

for _p in ("/opt/trn_rl_repo",):
    if _p not in sys.path:
        sys.path.insert(0, _p)

import ml_dtypes
import numpy as np

import concourse.bass as bass  # noqa: F401
import concourse.mybir as mybir
import concourse.tile as tile
from concourse import bacc
from concourse.bass_utils import run_bass_kernel_spmd
from concourse.masks import make_identity

T, E = 4096, 1024
H, D = 16, 64
NCORES = 8
HPC = H // NCORES          # heads per core = 2
HD = HPC * D               # hidden dims per core = 128
NT = T // 512              # 8 tq blocks of 512
NE = E // 128              # 8 e-chunks of 128
NTB = T // 128             # 32 tk blocks of 128

F32 = mybir.dt.float32
BF16 = mybir.dt.bfloat16
NPBF16 = np.dtype(ml_dtypes.bfloat16)
AF = mybir.ActivationFunctionType


def _build_kernel():
    nc = bacc.Bacc("TRN2", target_bir_lowering=False, debug=False)

    xT = nc.dram_tensor("xT", [E, T], BF16, kind="ExternalInput")
    wq = nc.dram_tensor("wq", [E, HD], BF16, kind="ExternalInput")
    wk = nc.dram_tensor("wk", [E, HD], BF16, kind="ExternalInput")
    wv = nc.dram_tensor("wv", [E, HD], BF16, kind="ExternalInput")
    bqkv = nc.dram_tensor("bqkv", [3, HD, 1], F32, kind="ExternalInput")
    wo = nc.dram_tensor("wo", [HD, E], BF16, kind="ExternalInput")
    out = nc.dram_tensor("out", [T, E], BF16, kind="ExternalOutput")

    with tile.TileContext(nc) as tc:
        _body(nc, tc, xT, wq, wk, wv, bqkv, wo, out)
    nc.compile()
    return nc


def _body(nc, tc, xT, wq, wk, wv, bqkv, wo, out):
    from contextlib import ExitStack

    ctx = ExitStack()
    with ctx:
        const = ctx.enter_context(tc.tile_pool(name="const", bufs=1))
        big = ctx.enter_context(tc.tile_pool(name="big", bufs=1))
        xpool = ctx.enter_context(tc.tile_pool(name="xp", bufs=3))
        ppool = ctx.enter_context(tc.tile_pool(name="pp", bufs=4))
        opool = ctx.enter_context(tc.tile_pool(name="op", bufs=3))
        small = ctx.enter_context(tc.tile_pool(name="sm", bufs=4))
        ps_mm = ctx.enter_context(tc.tile_pool(name="ps_mm", bufs=2, space="PSUM"))
        ps_o = ctx.enter_context(tc.tile_pool(name="ps_o", bufs=2, space="PSUM"))
        ps_q = ctx.enter_context(tc.tile_pool(name="ps_q", bufs=2, space="PSUM"))

        # ---- constants / weights ----
        # scratch read by the clock-warming matmuls; one cheap memset is
        # its only producer so the warm-up can start immediately
        warm_src = const.tile([128, 512], BF16)
        nc.vector.memset(warm_src[:], 0.0)
        identb = const.tile([128, 128], BF16)
        make_identity(nc, identb[:])

        xs_map = {}

        def load_x(tcc):
            # one DMA for the whole [1024, 512] chunk (8 e-slices)
            ts512 = slice(tcc * 512, (tcc + 1) * 512)
            xsb = xpool.tile([128, NE, 512], BF16, tag="xsb")
            nc.sync.dma_start(
                xsb[:], xT[:, ts512].rearrange("(a p) t -> p a t", p=128)
            )
            xs_map[tcc] = xsb

        # x chunk 0 first: it gates the first QKV matmuls, while the
        # weight DMAs can hide under the warm-up/projection stream
        load_x(0)

        wq_sb = const.tile([128, NE, HD], BF16)
        wk_sb = const.tile([128, NE, HD], BF16)
        wv_sb = const.tile([128, NE, HD], BF16)
        for w_dram, w_sb in ((wq, wq_sb), (wk, wk_sb), (wv, wv_sb)):
            nc.sync.dma_start(
                w_sb[:], w_dram[:].rearrange("(a p) c -> p a c", p=128)
            )
        bq_sb = const.tile([128, 1], F32)
        bk_sb = const.tile([128, 1], F32)
        bv_sb = const.tile([128, 1], F32)
        nc.sync.dma_start(bq_sb[:], bqkv[0])
        nc.sync.dma_start(bk_sb[:], bqkv[1])
        nc.sync.dma_start(bv_sb[:], bqkv[2])

        load_x(1)
        wo_sb = const.tile([128, E], BF16)
        nc.sync.dma_start(wo_sb[:], wo[:])

        qT = big.tile([128, T], BF16)
        kT = big.tile([128, T], BF16)
        vT = big.tile([128, T], BF16)
        # V row-major per (tk block, head) plus a ones column.
        V2 = big.tile([128, NTB, HPC, D + 1], BF16)
        # normalized attention outputs, transposed: rows h*64+d, cols t
        UnT = big.tile([128, T], BF16)

        nc.gpsimd.memset(V2[:, :, :, D], 1.0)

        wparams = ((wq_sb, bq_sb), (wk_sb, bk_sb), (wv_sb, bv_sb))
        qkv_dst = (qT, kT, vT)

        def emit_qkv(tcc, m):
            w_sb, b_sb = wparams[m]
            ts512 = slice(tcc * 512, (tcc + 1) * 512)
            ps = ps_q.tile([128, 512], F32, tag="q")  # noqa: same pool as outproj
            for ec in range(NE):
                nc.tensor.matmul(
                    ps[:], w_sb[:, ec, :], xs_map[tcc][:, ec, :],
                    start=(ec == 0), stop=(ec == NE - 1),
                )
            nc.vector.tensor_scalar_add(qkv_dst[m][:, ts512], ps[:], b_sb[:])

        def emit_vtrans(tcc, j):
            # V' transpose, K=128 full-width: both heads in one go
            tb = 4 * tcc + j
            pst = ps_q.tile([128, 128], BF16, tag="q")
            nc.tensor.transpose(
                pst[:], vT[:, tb * 128:(tb + 1) * 128], identb[:]
            )
            nc.vector.tensor_copy(
                V2[:, tb, :, 0:D],
                pst[:].rearrange("p (h d) -> p h d", h=HPC),
            )

        def emit_piece(piece):
            kind = piece[0]
            if kind == "qkv":
                emit_qkv(piece[1], piece[2])
            elif kind == "vtrans":
                emit_vtrans(piece[1], piece[2])
            else:
                _outproj_tile(nc, ps_q, opool, UnT, wo_sb, out, piece[1])

        def emit_S(qb, tb):
            # row-tiled pair: head h contracts its own 64 dims on array
            # rows h*64..h*64+63; both MMs stream concurrently
            f0 = max(0, tb * 128 - qb * 512)
            psS = ps_mm.tile([128, HPC, 512], F32, tag="mm")
            t0 = qb * 512 + f0
            t1 = (qb + 1) * 512
            for h in range(HPC):
                nc.tensor.matmul(
                    psS[:, h, f0:512],
                    kT[h * D:(h + 1) * D, tb * 128:(tb + 1) * 128],
                    qT[h * D:(h + 1) * D, t0:t1],
                    start=True, stop=True,
                )
            return psS

        # ---- prologue: x for chunks 0/1, full QKV chunk 0. Dummy
        # identity matmuls fill the input-DMA wait: they cost nothing the
        # PE would otherwise use, and ~4us of sustained full-array
        # activity flips the clock gate to 2.4 GHz before the real
        # matmuls start ----
        for i in range(10):
            wps = ps_q.tile([128, 512], F32, tag="q")
            nc.tensor.matmul(wps[:], warm_src[:, 0:128], warm_src[:],
                             start=True, stop=True)
        for m in range(3):
            emit_qkv(0, m)
        for j in range(4):
            emit_vtrans(0, j)

        # ---- merged pipeline: per step, the attention q block `step`
        # runs its ACT-paced tk loop while the NEXT chunk's projection
        # matmuls and the lag-1 out-proj are spread through it as PE
        # filler pieces ----
        for step in range(NT):
            if step + 2 < NT:
                load_x(step + 2)
            pieces = []
            if step + 1 < NT:
                pieces += [("qkv", step + 1, m) for m in range(3)]
                pieces += [("vtrans", step + 1, j) for j in range(4)]
            if step >= 1:
                # lag-1: block step-1's normalize settles a few tk
                # iterations into this step
                pieces += [("out", tt)
                           for tt in range((step - 1) * 4, step * 4)]
            qb = step
            nblk = 4 * (qb + 1)
            # out pieces go in the second half of the step: their UnT
            # stationary waits on the previous block's normalize chain,
            # and the in-order PE queue would stall every matmul behind
            # them if they were issued early
            emit_at = {}
            outs = [p for p in pieces if p[0] == "out"]
            rest = [p for p in pieces if p[0] != "out"]
            for i, piece in enumerate(rest):
                emit_at.setdefault((i + 1) * nblk // (len(rest) + 1),
                                   []).append(piece)
            lo = nblk // 2
            for i, piece in enumerate(outs):
                emit_at.setdefault(lo + (i + 1) * (nblk - lo) // (len(outs) + 1),
                                   []).append(piece)
            pos = []
            for h in range(HPC):
                po = ps_o.tile([D + 1, 512], F32, tag="o")
                pos.append(po)

            def emit_O(tb, P):
                f0 = max(0, tb * 128 - qb * 512)
                for h in range(HPC):
                    nc.tensor.matmul(
                        pos[h][:, f0:512],
                        V2[:, tb, h, :],
                        P[:, h, f0:512],
                        start=(tb == 0), stop=(tb == nblk - 1),
                    )

            Stiles = {0: emit_S(qb, 0)}
            Pprev = None
            for tb in range(nblk):
                diag = tb >= 4 * qb
                f0 = max(0, tb * 128 - qb * 512)
                psS = Stiles.pop(tb)
                P = ppool.tile([128, HPC, 512], BF16, tag="P")
                if f0 == 0:
                    # flat 1D AP — ~100ns faster on ACT than the strided view
                    nc.scalar.activation(
                        P[:].rearrange("p h f -> p (h f)"),
                        psS[:].rearrange("p h f -> p (h f)"),
                        AF.Exp, scale=0.125,
                    )
                else:
                    nc.scalar.activation(
                        P[:, :, f0:512], psS[:, :, f0:512], AF.Exp, scale=0.125
                    )
                if diag:
                    # keep where tq >= tk:
                    # (qb*512 + f0 + f') - (tb*128 + p) >= 0
                    nc.gpsimd.affine_select(
                        out=P[:, :, f0:512], in_=P[:, :, f0:512],
                        compare_op=mybir.AluOpType.is_ge,
                        fill=0.0,
                        base=qb * 512 + f0 - tb * 128,
                        channel_multiplier=-1,
                        pattern=[[0, HPC], [1, 512 - f0]],
                    )
                # PE stream per iteration: S(tb+1), filler pieces, then
                # O'(tb-1) — whose exp finished an iteration ago, so the
                # PE never sits out an exp latency
                if tb + 1 < nblk:
                    Stiles[tb + 1] = emit_S(qb, tb + 1)
                for piece in emit_at.get(tb, ()):
                    emit_piece(piece)
                if Pprev is not None:
                    emit_O(tb - 1, Pprev)
                Pprev = P
            emit_O(nblk - 1, Pprev)
            # normalize: U = O'[0:64] * (1 / O'[64]) columnwise; the two
            # heads' chains are interleaved by stage so DVE and GpSimd
            # pipeline them instead of running them back-to-back
            drows, rbs, rbrs = [], [], []
            for h in range(HPC):
                drow = small.tile([1, 512], F32, tag="drow")
                nc.vector.tensor_copy(drow[:], pos[h][D:D + 1, :])
                drows.append(drow)
            for h in range(HPC):
                rb = small.tile([D, 512], F32, tag="rb")
                nc.gpsimd.partition_broadcast(rb[:], drows[h][:], channels=D)
                rbs.append(rb)
            for h in range(HPC):
                rbr = small.tile([D, 512], F32, tag="rbr")
                nc.vector.reciprocal_approx_fast(rbr[:], rbs[h][:])
                rbrs.append(rbr)
            for h in range(HPC):
                nc.vector.tensor_mul(
                    UnT[h * D:(h + 1) * D, qb * 512:(qb + 1) * 512],
                    pos[h][0:D, :], rbrs[h][:],
                )
        # keep the PE clock warm through the last normalize chain so the
        # epilogue out-proj runs at 2.4 GHz
        for i in range(20):
            wps = ps_q.tile([128, 512], F32, tag="q")
            nc.tensor.matmul(wps[:], warm_src[:, 0:128], warm_src[:],
                             start=True, stop=True)
        for tt in range((NT - 1) * 4, NT * 4):
            _outproj_tile(nc, ps_q, opool, UnT, wo_sb, out, tt)


def _outproj_tile(nc, ps_q, opool, UnT, wo_sb, out, tt):
    osb2 = opool.tile([128, E], BF16, tag="out")
    for half in range(2):
        psc = ps_q.tile([128, 512], F32, tag="q")
        nc.tensor.matmul(
            psc[:],
            UnT[:, tt * 128:(tt + 1) * 128],
            wo_sb[:, half * 512:(half + 1) * 512],
            start=True, stop=True,
        )
        nc.vector.tensor_copy(
            osb2[:, half * 512:(half + 1) * 512], psc[:]
        )
    nc.sync.dma_start(out[tt * 128:(tt + 1) * 128, :], osb2[:])


_NC_CACHE = None


def _get_nc():
    global _NC_CACHE
    if _NC_CACHE is None:
        _NC_CACHE = _build_kernel()
    return _NC_CACHE


def _make_in_maps(x, w_qkv, b_qkv, w_out):
    x2 = np.asarray(x, dtype=np.float32).reshape(T, E)
    xT = np.ascontiguousarray(x2.T).astype(NPBF16)
    w_qkv = np.asarray(w_qkv, dtype=np.float32)
    b_qkv = np.asarray(b_qkv, dtype=np.float32)
    w_out = np.asarray(w_out, dtype=np.float32)
    in_maps = []
    for c in range(NCORES):
        s = slice(c * HD, (c + 1) * HD)
        in_maps.append({
            "xT": xT,
            "wq": np.ascontiguousarray(
                w_qkv[:, 0 * E + c * HD:0 * E + (c + 1) * HD]).astype(NPBF16),
            "wk": np.ascontiguousarray(
                w_qkv[:, 1 * E + c * HD:1 * E + (c + 1) * HD]).astype(NPBF16),
            "wv": np.ascontiguousarray(
                w_qkv[:, 2 * E + c * HD:2 * E + (c + 1) * HD]).astype(NPBF16),
            "bqkv": np.ascontiguousarray(
                np.stack([
                    b_qkv[0 * E + c * HD:0 * E + (c + 1) * HD],
                    b_qkv[1 * E + c * HD:1 * E + (c + 1) * HD],
                    b_qkv[2 * E + c * HD:2 * E + (c + 1) * HD],
                ]).reshape(3, HD, 1)
            ),
            "wo": np.ascontiguousarray(w_out[s, :]).astype(NPBF16),
        })
    return in_maps


def run_sharded(x, w_qkv, b_qkv, w_out, b_out, trace=False):
    """Run the SPMD kernel; returns (full_output, BassKernelResults)."""
    nc = _get_nc()
    in_maps = _make_in_maps(x, w_qkv, b_qkv, w_out)
    res = run_bass_kernel_spmd(
        nc, in_maps, core_ids=list(range(NCORES)), trace=trace
    )
    acc = np.zeros((T, E), dtype=np.float32)
    for c in range(NCORES):
        acc += np.asarray(res.results[c]["out"], dtype=np.float32)
    acc += np.asarray(b_out, dtype=np.float32)[None, :]
    return acc.reshape(1, T, E), res


def kernel(x, w_qkv, b_qkv, w_out, b_out):
    out, _ = run_sharded(x, w_qkv, b_qkv, w_out, b_out, trace=False)
    return out


# revision 27
# speedup vs baseline: 1.2708x; 1.0832x over previous
"""Causal self-attention (B=1, T=4096, E=1024, H=16, D=64) on 8 TRN2 NeuronCores.

Sharding: tensor-parallel over heads — each core owns 2 heads (128 of the
1024 hidden dims). Each core computes its slice of the QKV projection, a
flash-style causal attention for its 2 heads, and a partial output
projection (rows of w_out for its head dims). The host sums the 8 partial
outputs (the row-parallel all-reduce) and adds b_out.

Matmul inputs are bf16 (1 cycle/row on the PE), accumulation fp32 in PSUM.
x and weights are converted to bf16 on the host (halves their DMA too).

The per-head d=64 S^T matmuls use 64x128 PE row tiling: head 0 runs on
array rows 0-63 (tile (0,0)) while head 1 runs concurrently on rows 64-127
(tile (64,0)), each contracting its own K=64 head dims. The two MMs issue
back-to-back and stream in parallel, so a [tk=128, 2, 512] S^T block costs
~512 PE cycles instead of the 1024 a zero-padded K=128 packing costs.
qT/kT/vT all live in the natural [128 = 2 heads x 64 dims, T] layout.

Per-core dataflow (feature-major throughout; tq blocks of 512; tk blocks
of 128):
  kT/vT/qT [128, 4096]  (phase A, K=e chunks of 128)
  V' [tk, 2, 65] = PE-transpose of vT + ones column
  per (512-wide tq block qb):
    per tk block tb:   S^T[tk, h, f] row-tiled pair     [128, 2, 512] PSUM
                       P = exp(0.125 * S^T)             ACT, PSUM->SBUF bf16
                       (diag blocks: affine_select zeroes tq < tk)
      per head h:      O'_h += V'_h.T @ P[:, h, :]      [65, 512] PSUM accum
    row 64 of O'_h = softmax denominators (ones column trick);
    normalize: broadcast denom row, fast reciprocal, columnwise scale
    -> UnT [hd=128, t] (both heads stacked)
  out_partial[t,:] = UnT_tile.T @ w_out_rows  (pipelined one qb behind,
  bf16 partials; host upcasts and sums)
"""

import sys

for _p in ("/opt/trn_rl_repo",):
    if _p not in sys.path:
        sys.path.insert(0, _p)

import ml_dtypes
import numpy as np

import concourse.bass as bass  # noqa: F401
import concourse.mybir as mybir
import concourse.tile as tile
from concourse import bacc
from concourse.bass_utils import run_bass_kernel_spmd
from concourse.masks import make_identity

T, E = 4096, 1024
H, D = 16, 64
NCORES = 8
HPC = H // NCORES          # heads per core = 2
HD = HPC * D               # hidden dims per core = 128
NT = T // 512              # 8 tq blocks of 512
NE = E // 128              # 8 e-chunks of 128
NTB = T // 128             # 32 tk blocks of 128

F32 = mybir.dt.float32
BF16 = mybir.dt.bfloat16
NPBF16 = np.dtype(ml_dtypes.bfloat16)
AF = mybir.ActivationFunctionType


def _build_kernel():
    nc = bacc.Bacc("TRN2", target_bir_lowering=False, debug=False)

    xT = nc.dram_tensor("xT", [E, T], BF16, kind="ExternalInput")
    wq = nc.dram_tensor("wq", [E, HD], BF16, kind="ExternalInput")
    wk = nc.dram_tensor("wk", [E, HD], BF16, kind="ExternalInput")
    wv = nc.dram_tensor("wv", [E, HD], BF16, kind="ExternalInput")
    bqkv = nc.dram_tensor("bqkv", [3, HD, 1], F32, kind="ExternalInput")
    wo = nc.dram_tensor("wo", [HD, E], BF16, kind="ExternalInput")
    out = nc.dram_tensor("out", [T, E], BF16, kind="ExternalOutput")

    with tile.TileContext(nc) as tc:
        _body(nc, tc, xT, wq, wk, wv, bqkv, wo, out)
    nc.compile()
    return nc


def _body(nc, tc, xT, wq, wk, wv, bqkv, wo, out):
    from contextlib import ExitStack

    ctx = ExitStack()
    with ctx:
        const = ctx.enter_context(tc.tile_pool(name="const", bufs=1))
        big = ctx.enter_context(tc.tile_pool(name="big", bufs=1))
        xpool = ctx.enter_context(tc.tile_pool(name="xp", bufs=3))
        ppool = ctx.enter_context(tc.tile_pool(name="pp", bufs=4))
        opool = ctx.enter_context(tc.tile_pool(name="op", bufs=3))
        small = ctx.enter_context(tc.tile_pool(name="sm", bufs=4))
        ps_mm = ctx.enter_context(tc.tile_pool(name="ps_mm", bufs=2, space="PSUM"))
        ps_o = ctx.enter_context(tc.tile_pool(name="ps_o", bufs=2, space="PSUM"))
        ps_q = ctx.enter_context(tc.tile_pool(name="ps_q", bufs=2, space="PSUM"))

        # ---- constants / weights ----
        # scratch read by the clock-warming matmuls; one cheap memset is
        # its only producer so the warm-up can start immediately
        warm_src = const.tile([128, 512], BF16)
        nc.vector.memset(warm_src[:], 0.0)
        identb = const.tile([128, 128], BF16)
        make_identity(nc, identb[:])

        xs_map = {}

        def load_x(tcc):
            # one DMA for the whole [1024, 512] chunk (8 e-slices)
            ts512 = slice(tcc * 512, (tcc + 1) * 512)
            xsb = xpool.tile([128, NE, 512], BF16, tag="xsb")
            nc.sync.dma_start(
                xsb[:], xT[:, ts512].rearrange("(a p) t -> p a t", p=128)
            )
            xs_map[tcc] = xsb

        # x chunk 0 first: it gates the first QKV matmuls, while the
        # weight DMAs can hide under the warm-up/projection stream
        load_x(0)

        # q/k weights + biases right behind x0 — they gate the first S
        # block; v, x1, wo can trail
        wq_sb = const.tile([128, NE, HD], BF16)
        wk_sb = const.tile([128, NE, HD], BF16)
        wv_sb = const.tile([128, NE, HD], BF16)
        bq_sb = const.tile([128, 1], F32)
        bk_sb = const.tile([128, 1], F32)
        bv_sb = const.tile([128, 1], F32)
        for w_dram, w_sb in ((wq, wq_sb), (wk, wk_sb)):
            nc.sync.dma_start(
                w_sb[:], w_dram[:].rearrange("(a p) c -> p a c", p=128)
            )
        nc.sync.dma_start(bq_sb[:], bqkv[0])
        nc.sync.dma_start(bk_sb[:], bqkv[1])
        nc.sync.dma_start(
            wv_sb[:], wv[:].rearrange("(a p) c -> p a c", p=128)
        )
        nc.sync.dma_start(bv_sb[:], bqkv[2])

        load_x(1)
        wo_sb = const.tile([128, E], BF16)
        nc.sync.dma_start(wo_sb[:], wo[:])

        qT = big.tile([128, T], BF16)
        kT = big.tile([128, T], BF16)
        vT = big.tile([128, T], BF16)
        # V row-major per (tk block, head) plus a ones column.
        V2 = big.tile([128, NTB, HPC, D + 1], BF16)
        # normalized attention outputs, transposed: rows h*64+d, cols t
        UnT = big.tile([128, T], BF16)

        nc.gpsimd.memset(V2[:, :, :, D], 1.0)

        wparams = ((wq_sb, bq_sb), (wk_sb, bk_sb), (wv_sb, bv_sb))
        qkv_dst = (qT, kT, vT)

        def emit_qkv(tcc, m):
            w_sb, b_sb = wparams[m]
            ts512 = slice(tcc * 512, (tcc + 1) * 512)
            ps = ps_q.tile([128, 512], F32, tag="q")  # noqa: same pool as outproj
            for ec in range(NE):
                nc.tensor.matmul(
                    ps[:], w_sb[:, ec, :], xs_map[tcc][:, ec, :],
                    start=(ec == 0), stop=(ec == NE - 1),
                )
            nc.vector.tensor_scalar_add(qkv_dst[m][:, ts512], ps[:], b_sb[:])

        def emit_vtrans(tcc, j):
            # V' transpose, K=128 full-width: both heads in one go
            tb = 4 * tcc + j
            pst = ps_q.tile([128, 128], BF16, tag="q")
            nc.tensor.transpose(
                pst[:], vT[:, tb * 128:(tb + 1) * 128], identb[:]
            )
            nc.vector.tensor_copy(
                V2[:, tb, :, 0:D],
                pst[:].rearrange("p (h d) -> p h d", h=HPC),
            )

        def emit_piece(piece):
            kind = piece[0]
            if kind == "qkv":
                emit_qkv(piece[1], piece[2])
            elif kind == "vtrans":
                emit_vtrans(piece[1], piece[2])
            else:
                _outproj_tile(nc, ps_q, opool, UnT, wo_sb, out, piece[1])

        def emit_S(qb, tb):
            # row-tiled pair: head h contracts its own 64 dims on array
            # rows h*64..h*64+63; both MMs stream concurrently
            f0 = max(0, tb * 128 - qb * 512)
            psS = ps_mm.tile([128, HPC, 512], F32, tag="mm")
            t0 = qb * 512 + f0
            t1 = (qb + 1) * 512
            for h in range(HPC):
                nc.tensor.matmul(
                    psS[:, h, f0:512],
                    kT[h * D:(h + 1) * D, tb * 128:(tb + 1) * 128],
                    qT[h * D:(h + 1) * D, t0:t1],
                    start=True, stop=True,
                )
            return psS

        # ---- prologue: x for chunks 0/1, full QKV chunk 0. Dummy
        # identity matmuls fill the input-DMA wait: they cost nothing the
        # PE would otherwise use, and ~4us of sustained full-array
        # activity flips the clock gate to 2.4 GHz before the real
        # matmuls start ----
        for i in range(14):
            wps = ps_q.tile([128, 512], F32, tag="q")
            nc.tensor.matmul(wps[:], warm_src[:, 0:128], warm_src[:],
                             start=True, stop=True)
        # only q/k here — they gate the first S block; the v projection
        # and V' transposes ride as block-0 pieces (each vtrans(0,j) is
        # needed only by O'(tb=j), one iteration later)
        emit_qkv(0, 0)
        emit_qkv(0, 1)

        # ---- merged pipeline: per step, the attention q block `step`
        # runs its ACT-paced tk loop while the NEXT chunk's projection
        # matmuls and the lag-1 out-proj are spread through it as PE
        # filler pieces ----
        Stiles = {}
        for step in range(NT):
            if step + 2 < NT:
                load_x(step + 2)
            pieces = []
            if step == 0:
                # chunk 0's v projection + V' transposes, deferred from
                # the prologue so the first exp starts sooner; the slot
                # spread puts vtrans(0,0) at slot 0, ahead of O'(0)
                pieces += [("qkv", 0, 2)]
                pieces += [("vtrans", 0, j) for j in range(4)]
            if step + 1 < NT:
                pieces += [("qkv", step + 1, m) for m in range(3)]
                pieces += [("vtrans", step + 1, j) for j in range(4)]
            if step >= 2:
                # lag-2: block step-2's normalize settled a whole step
                # ago, so these never stall the in-order PE queue
                pieces += [("out", tt)
                           for tt in range((step - 2) * 4, (step - 1) * 4)]
            late = []
            if step == NT - 1:
                # qb6's out-proj rides late in the final tk loop; its
                # normalize settles a few iterations in
                late += [("out", tt)
                         for tt in range((NT - 2) * 4, (NT - 1) * 4)]
            qb = step
            nblk = 4 * (qb + 1)
            emit_at = {}
            for i, piece in enumerate(pieces):
                emit_at.setdefault((i + 1) * nblk // (len(pieces) + 1),
                                   []).append(piece)
            lo = nblk // 2
            for i, piece in enumerate(late):
                emit_at.setdefault(lo + (i + 1) * (nblk - lo) // (len(late) + 1),
                                   []).append(piece)
            pos = []
            for h in range(HPC):
                po = ps_o.tile([D + 1, 512], F32, tag="o")
                pos.append(po)

            def emit_O(tb, P):
                f0 = max(0, tb * 128 - qb * 512)
                for h in range(HPC):
                    nc.tensor.matmul(
                        pos[h][:, f0:512],
                        V2[:, tb, h, :],
                        P[:, h, f0:512],
                        start=(tb == 0), stop=(tb == nblk - 1),
                    )

            if (qb, 0) not in Stiles:
                Stiles[(qb, 0)] = emit_S(qb, 0)
            Pprev = None
            for tb in range(nblk):
                diag = tb >= 4 * qb
                f0 = max(0, tb * 128 - qb * 512)
                psS = Stiles.pop((qb, tb))
                P = ppool.tile([128, HPC, 512], BF16, tag="P")
                if f0 == 0:
                    # flat 1D AP — ~100ns faster on ACT than the strided view
                    nc.scalar.activation(
                        P[:].rearrange("p h f -> p (h f)"),
                        psS[:].rearrange("p h f -> p (h f)"),
                        AF.Exp, scale=0.125,
                    )
                else:
                    nc.scalar.activation(
                        P[:, :, f0:512], psS[:, :, f0:512], AF.Exp, scale=0.125
                    )
                if diag:
                    # keep where tq >= tk:
                    # (qb*512 + f0 + f') - (tb*128 + prt) >= 0
                    nc.gpsimd.affine_select(
                        out=P[:, :, f0:512], in_=P[:, :, f0:512],
                        compare_op=mybir.AluOpType.is_ge,
                        fill=0.0,
                        base=qb * 512 + f0 - tb * 128,
                        channel_multiplier=-1,
                        pattern=[[0, HPC], [1, 512 - f0]],
                    )
                # PE stream per iteration: S(tb+1), filler pieces, then
                # O'(tb-1) — whose exp finished an iteration ago, so the
                # PE never sits out an exp latency
                if tb + 1 < nblk:
                    Stiles[(qb, tb + 1)] = emit_S(qb, tb + 1)
                elif qb + 1 < NT:
                    # cross-boundary: the next block's first S goes out
                    # before this block's last O, so its exp starts
                    # while the boundary normalize runs
                    Stiles[(qb + 1, 0)] = emit_S(qb + 1, 0)
                for piece in emit_at.get(tb, ()):
                    emit_piece(piece)
                if Pprev is not None:
                    emit_O(tb - 1, Pprev)
                Pprev = P
            emit_O(nblk - 1, Pprev)
            # normalize: U = O'[0:64] * (1 / O'[64]) columnwise; the two
            # heads' chains are interleaved by stage so DVE and GpSimd
            # pipeline them instead of running them back-to-back
            # evacuate O' to SBUF right away: this releases the two pos
            # PSUM banks (which the NEXT block's first O matmul reuses)
            # after ~1.5us instead of holding them through the whole
            # normalize chain
            posb, rbs, rbrs = [], [], []
            for h in range(HPC):
                pb = small.tile([D + 1, 512], F32, tag="posb")
                nc.vector.tensor_copy(pb[:], pos[h][:])
                posb.append(pb)
            drows = []
            for h in range(HPC):
                drow = small.tile([1, 512], F32, tag="drow")
                nc.vector.tensor_copy(drow[:], posb[h][D:D + 1, :])
                drows.append(drow)
            for h in range(HPC):
                rb = small.tile([D, 512], F32, tag="rb")
                nc.gpsimd.partition_broadcast(rb[:], drows[h][:], channels=D)
                rbs.append(rb)
            for h in range(HPC):
                rbr = small.tile([D, 512], F32, tag="rbr")
                nc.vector.reciprocal_approx_fast(rbr[:], rbs[h][:])
                rbrs.append(rbr)
            for h in range(HPC):
                nc.vector.tensor_mul(
                    UnT[h * D:(h + 1) * D, qb * 512:(qb + 1) * 512],
                    posb[h][0:D, :], rbrs[h][:],
                )
        # keep the PE clock warm through the last normalize chain so the
        # epilogue out-proj runs at 2.4 GHz
        for i in range(32):
            wps = ps_q.tile([128, 512], F32, tag="q")
            nc.tensor.matmul(wps[:], warm_src[:, 0:128], warm_src[:],
                             start=True, stop=True)
        for tt in range((NT - 1) * 4, NT * 4):
            _outproj_tile(nc, ps_q, opool, UnT, wo_sb, out, tt)


def _outproj_tile(nc, ps_q, opool, UnT, wo_sb, out, tt):
    osb2 = opool.tile([128, E], BF16, tag="out")
    for half in range(2):
        psc = ps_q.tile([128, 512], F32, tag="q")
        nc.tensor.matmul(
            psc[:],
            UnT[:, tt * 128:(tt + 1) * 128],
            wo_sb[:, half * 512:(half + 1) * 512],
            start=True, stop=True,
        )
        nc.vector.tensor_copy(
            osb2[:, half * 512:(half + 1) * 512], psc[:]
        )
    nc.sync.dma_start(out[tt * 128:(tt + 1) * 128, :], osb2[:])


_NC_CACHE = None


def _get_nc():
    global _NC_CACHE
    if _NC_CACHE is None:
        _NC_CACHE = _build_kernel()
    return _NC_CACHE


def _make_in_maps(x, w_qkv, b_qkv, w_out):
    x2 = np.asarray(x, dtype=np.float32).reshape(T, E)
    xT = np.ascontiguousarray(x2.T).astype(NPBF16)
    w_qkv = np.asarray(w_qkv, dtype=np.float32)
    b_qkv = np.asarray(b_qkv, dtype=np.float32)
    w_out = np.asarray(w_out, dtype=np.float32)
    in_maps = []
    for c in range(NCORES):
        s = slice(c * HD, (c + 1) * HD)
        in_maps.append({
            "xT": xT,
            "wq": np.ascontiguousarray(
                w_qkv[:, 0 * E + c * HD:0 * E + (c + 1) * HD]).astype(NPBF16),
            "wk": np.ascontiguousarray(
                w_qkv[:, 1 * E + c * HD:1 * E + (c + 1) * HD]).astype(NPBF16),
            "wv": np.ascontiguousarray(
                w_qkv[:, 2 * E + c * HD:2 * E + (c + 1) * HD]).astype(NPBF16),
            "bqkv": np.ascontiguousarray(
                np.stack([
                    b_qkv[0 * E + c * HD:0 * E + (c + 1) * HD],
                    b_qkv[1 * E + c * HD:1 * E + (c + 1) * HD],
                    b_qkv[2 * E + c * HD:2 * E + (c + 1) * HD],
                ]).reshape(3, HD, 1)
            ),
            "wo": np.ascontiguousarray(w_out[s, :]).astype(NPBF16),
        })
    return in_maps


def run_sharded(x, w_qkv, b_qkv, w_out, b_out, trace=False):
    """Run the SPMD kernel; returns (full_output, BassKernelResults)."""
    nc = _get_nc()
    in_maps = _make_in_maps(x, w_qkv, b_qkv, w_out)
    res = run_bass_kernel_spmd(
        nc, in_maps, core_ids=list(range(NCORES)), trace=trace
    )
    acc = np.zeros((T, E), dtype=np.float32)
    for c in range(NCORES):
        acc += np.asarray(res.results[c]["out"], dtype=np.float32)
    acc += np.asarray(b_out, dtype=np.float32)[None, :]
    return acc.reshape(1, T, E), res


def kernel(x, w_qkv, b_qkv, w_out, b_out):
    out, _ = run_sharded(x, w_qkv, b_qkv, w_out, b_out, trace=False)
    return out
